# revision 9
# baseline (speedup 1.0000x reference)
"""AnomalyDAE base forward on 8 Trainium2 NeuronCores.

Strategy (nodes row-sharded, 1280 per core, N padded 10000 -> 10240):
  - structure encoder (h = relu(x@Ws^T), g = h@Wg^T, attention logits a_s/a_d)
    computed per-core on local rows in feature-major layout [64, nodes].
  - AllGather #1 ships node-major g (bf16) + node-major a_s to every core.
  - GAT aggregation per core over its destination rows: for each 128-src
    block, a sparse [128 src, 1280 dst] matrix R of softmax numerators is
    built with gpsimd local_scatter from per-edge exp(leaky_relu(a_s+a_d))
    values, then PE accumulates [g|1]^T @ R into PSUM -> weighted sums and
    softmax denominators in one stream.
  - AllGather #2 ships emb (bf16); s_ = sigmoid(emb_local @ emb_full^T) is a
    row-sharded [1280, 10240] matmul + ACT sigmoid, streamed to HBM.
  - attribute path: per-core partial of x^T @ W_a1^T, AllReduce, tiny matmuls,
    x_ = emb @ a2^T row-sharded.
"""

import os
import sys

import numpy as np

sys.path.insert(0, "/opt/trn_rl_repo")

N = 10000
IN = 256
EMB = 64
HID = 64
NCORES = 8
NPAD = 10240
RPC = NPAD // NCORES  # 1280 rows per core
NCH = RPC // 128      # 10 node chunks per core
SB = NPAD // 128      # 80 global source blocks
NEG = 0.2

_CACHE = {}


def _host_edge_prep(edge_index):
    """Dedup edges (+self loops), assign to (core, group, src-block, slot).

    Returns W (slots per (group, block)), and per-core index/value tensors:
      ad_idx [8, 128, L//16] int16  - ap_gather indices (dst local), wrapped
      sc_idx [8, 128, L]     int16  - local_scatter indices (dst local or -1)
      c1     [8, 128, L]     f32    - edge multiplicity at valid slots
    """
    src = np.asarray(edge_index[0]).astype(np.int64)
    dst = np.asarray(edge_index[1]).astype(np.int64)
    loops = np.arange(N, dtype=np.int64)
    src = np.concatenate([src, loops])
    dst = np.concatenate([dst, loops])
    key = dst * 16384 + src
    uk, cnt = np.unique(key, return_counts=True)
    du = uk // 16384
    su = uk % 16384
    core = du // RPC
    dloc = du % RPC
    p = su % 128
    g = p // 16
    sb = su // 128
    cell = (core * 8 + g) * SB + sb
    order = np.argsort(cell, kind="stable")
    cell_s = cell[order]
    _, first_idx, inv = np.unique(cell_s, return_index=True, return_inverse=True)
    j = np.arange(len(cell_s)) - first_idx[inv]
    W = int(j.max()) + 1
    W = ((W + 3) // 4) * 4
    L = SB * W
    i_pos = sb[order] * W + j
    co = core[order]
    go = g[order]
    po = p[order]
    dl = dloc[order].astype(np.int16)
    ct = cnt[order].astype(np.float32)
    ad_idx = np.zeros((NCORES, 128, L // 16), np.int16)
    sc_idx = np.full((NCORES, 128, L), -1, np.int16)
    c1 = np.ones((NCORES, 128, L), np.float32)
    ad_idx[co, 16 * go + (i_pos % 16), i_pos // 16] = dl
    sc_idx[co, po, i_pos] = dl
    c1[co, po, i_pos] = ct
    return W, L, ad_idx, sc_idx, c1


def _build(W, L, trace=False):
    import concourse.bacc as bacc
    import concourse.bass as bass
    import concourse.mybir as mybir
    import concourse.tile as tile

    f32 = mybir.dt.float32
    bf16 = mybir.dt.bfloat16
    i16 = mybir.dt.int16
    AF = mybir.ActivationFunctionType
    ALU = mybir.AluOpType
    AP = bass.AP

    nc = bacc.Bacc("TRN2", target_bir_lowering=False, debug=False,
                   num_devices=NCORES)

    # ---- kernel I/O ----
    x_in = nc.dram_tensor("x_c", [RPC, IN], f32, kind="ExternalInput")
    wstru_in = nc.dram_tensor("W_struT", [IN, EMB], f32, kind="ExternalInput")
    bstru_in = nc.dram_tensor("b_stru", [EMB, 1], f32, kind="ExternalInput")
    wgat_in = nc.dram_tensor("W_gatT", [EMB, HID], f32, kind="ExternalInput")
    att2_in = nc.dram_tensor("att2", [HID, 2], f32, kind="ExternalInput")
    bgat_in = nc.dram_tensor("b_gat", [HID, 1], f32, kind="ExternalInput")
    wa1_in = nc.dram_tensor("W_a1T_c", [RPC, EMB], f32, kind="ExternalInput")
    ba1_in = nc.dram_tensor("b_a1", [EMB, 1], f32, kind="ExternalInput")
    wa2_in = nc.dram_tensor("W_a2T", [EMB, HID], f32, kind="ExternalInput")
    ba2_in = nc.dram_tensor("b_a2", [HID, 1], f32, kind="ExternalInput")
    ident_in = nc.dram_tensor("ident", [128, 128], f32, kind="ExternalInput")
    adidx_in = nc.dram_tensor("ad_idx", [128, L // 16], i16, kind="ExternalInput")
    scidx_in = nc.dram_tensor("sc_idx", [128, L], i16, kind="ExternalInput")
    c1_in = nc.dram_tensor("c1", [128, L], f32, kind="ExternalInput")

    out_s = nc.dram_tensor("out_s", [RPC, N], f32, kind="ExternalOutput")
    out_x = nc.dram_tensor("out_x", [RPC, IN], f32, kind="ExternalOutput")

    # collective bounce buffers (internal DRAM)
    AG1W = 656  # 640 g_nm cols + 10 as_nm cols + pad
    ag1_in = nc.dram_tensor("ag1_in", [128, AG1W], bf16)
    ag1_out = nc.dram_tensor("ag1_out", [128 * NCORES, AG1W], bf16,
                             addr_space="Shared")
    ar_in = nc.dram_tensor("ar_in", [EMB, IN], f32)
    ar_out = nc.dram_tensor("ar_out", [EMB, IN], f32, addr_space="Shared")
    ag2_in = nc.dram_tensor("ag2_in", [HID, RPC], bf16)
    ag2_out = nc.dram_tensor("ag2_out", [HID * NCORES, RPC], bf16,
                             addr_space="Shared")

    RG = [list(range(NCORES))]

    with tile.TileContext(nc) as tc:
        with (
            tc.tile_pool(name="persist", bufs=1) as pp,
            tc.tile_pool(name="psA", bufs=2, space="PSUM") as psA,
        ):
            # ---- persistent SBUF ----
            wstru = pp.tile([128, 2, EMB], f32)        # W_stru.T chunks
            bstru = pp.tile([EMB, 1], f32)
            wgat = pp.tile([EMB, HID], f32)
            att2 = pp.tile([HID, 2], f32)
            bgat = pp.tile([HID, 1], f32)
            wa1 = pp.tile([128, NCH, EMB], f32)
            ba1 = pp.tile([EMB, 1], f32)
            wa2 = pp.tile([EMB, HID], f32)
            ba2 = pp.tile([HID, 1], f32)
            ident = pp.tile([128, 128], f32)
            ones_row = pp.tile([1, 128], f32)
            h_T = pp.tile([EMB, RPC], f32)
            g_T = pp.tile([HID, RPC], f32)
            as_row = pp.tile([1, RPC], f32)
            ad_row = pp.tile([1, RPC], f32)
            as_nm_loc = pp.tile([128, NCH], bf16)
            g_nm_loc = pp.tile([128, NCH, HID], bf16)
            gones = pp.tile([128, SB, HID + 1], bf16)
            as_nm = pp.tile([128, SB], bf16)
            ad_rep = pp.tile([128, RPC], f32)
            emb_T = pp.tile([HID, RPC], f32)
            emb_bf = pp.tile([HID, RPC], bf16)
            a_part = pp.tile([EMB, IN], f32)
            a2_T = pp.tile([HID, IN], f32)
            denom_s = pp.tile([1, RPC], f32)
            denom_r = pp.tile([1, RPC], f32)
            recip_bc = pp.tile([HID, RPC], f32)

            # ---- load weights / constants ----
            nc.sync.dma_start(
                wstru[:], AP(wstru_in, 0, [[EMB, 128], [128 * EMB, 2], [1, EMB]]))
            nc.sync.dma_start(bstru[:], bstru_in[:])
            nc.sync.dma_start(wgat[:], wgat_in[:])
            nc.sync.dma_start(att2[:], att2_in[:])
            nc.sync.dma_start(bgat[:], bgat_in[:])
            nc.sync.dma_start(
                wa1[:], AP(wa1_in, 0, [[EMB, 128], [128 * EMB, NCH], [1, EMB]]))
            nc.sync.dma_start(ba1[:], ba1_in[:])
            nc.sync.dma_start(wa2[:], wa2_in[:])
            nc.sync.dma_start(ba2[:], ba2_in[:])
            nc.sync.dma_start(ident[:], ident_in[:])
            nc.vector.memset(ones_row[:], 1.0)
            nc.vector.memset(gones[:, :, HID], 1.0)

            with tc.tile_pool(name="phA", bufs=1) as pa:
                x_c = pa.tile([128, NCH, IN], f32)
                xT0 = pa.tile([128, RPC], f32)
                xT1 = pa.tile([128, RPC], f32)
                nc.sync.dma_start(
                    x_c[:], AP(x_in, 0, [[IN, 128], [128 * IN, NCH], [1, IN]]))

                # x^T via PE transposes
                for ch in range(NCH):
                    for half, xt in ((0, xT0), (1, xT1)):
                        pt = psA.tile([128, 128], f32, tag="pa")
                        nc.tensor.transpose(
                            pt[:], x_c[:, ch, half * 128:(half + 1) * 128],
                            ident[:])
                        nc.vector.tensor_copy(
                            xt[:, ch * 128:(ch + 1) * 128], pt[:])

                # attribute-path partial: a_part = sum_c wa1_c^T @ x_c
                pap = psA.tile([EMB, IN], f32, tag="pa")
                for ch in range(NCH):
                    nc.tensor.matmul(pap[:], lhsT=wa1[:, ch, :],
                                     rhs=x_c[:, ch, :],
                                     start=(ch == 0), stop=(ch == NCH - 1))
                nc.scalar.activation(a_part[:], pap[:], AF.Copy)
                nc.sync.dma_start(ar_in[:], a_part[:])

                # h^T = relu(W_stru @ x^T + b)
                for cs in range(3):
                    c0 = cs * 512
                    cw = min(512, RPC - c0)
                    ph = psA.tile([EMB, 512], f32, tag="pa")
                    nc.tensor.matmul(ph[:, :cw], lhsT=wstru[:, 0, :],
                                     rhs=xT0[:, c0:c0 + cw],
                                     start=True, stop=False)
                    nc.tensor.matmul(ph[:, :cw], lhsT=wstru[:, 1, :],
                                     rhs=xT1[:, c0:c0 + cw],
                                     start=False, stop=True)
                    nc.scalar.activation(h_T[:, c0:c0 + cw], ph[:, :cw],
                                         AF.Relu, bias=bstru[:])

            # g^T = W_gat @ h^T ; [a_s; a_d] = att2^T @ g^T
            for cs in range(3):
                c0 = cs * 512
                cw = min(512, RPC - c0)
                pg = psA.tile([HID, 512], f32, tag="pa")
                nc.tensor.matmul(pg[:, :cw], lhsT=wgat[:],
                                 rhs=h_T[:, c0:c0 + cw])
                nc.vector.tensor_copy(g_T[:, c0:c0 + cw], pg[:, :cw])
            for cs in range(3):
                c0 = cs * 512
                cw = min(512, RPC - c0)
                pas = psA.tile([1, 512], f32, tag="pa")
                nc.tensor.matmul(pas[:, :cw], lhsT=att2[:, 0:1],
                                 rhs=g_T[:, c0:c0 + cw])
                nc.vector.tensor_copy(as_row[:, c0:c0 + cw], pas[:, :cw])
                pad_ = psA.tile([1, 512], f32, tag="pa")
                nc.tensor.matmul(pad_[:, :cw], lhsT=att2[:, 1:2],
                                 rhs=g_T[:, c0:c0 + cw])
                nc.vector.tensor_copy(ad_row[:, c0:c0 + cw], pad_[:, :cw])

            # node-major g (bf16) via PE transposes; node-major a_s via
            # rank-1 matmuls (row chunk -> column)
            for ch in range(NCH):
                ptg = psA.tile([128, HID], f32, tag="pa")
                nc.tensor.transpose(
                    ptg[:], g_T[:, ch * 128:(ch + 1) * 128], ident[0:HID, 0:HID])
                nc.vector.tensor_copy(g_nm_loc[:, ch, :], ptg[:])
                pta = psA.tile([128, 1], f32, tag="pa")
                nc.tensor.matmul(pta[:], lhsT=as_row[0:1, ch * 128:(ch + 1) * 128],
                                 rhs=ones_row[0:1, 0:1])
                nc.vector.tensor_copy(as_nm_loc[:, ch:ch + 1], pta[:])

            # ad replicated across partitions via rank-1 PE broadcast
            for cs in range(3):
                c0 = cs * 512
                cw = min(512, RPC - c0)
                par = psA.tile([128, 512], f32, tag="pa")
                nc.tensor.matmul(par[:, :cw], lhsT=ones_row[:],
                                 rhs=ad_row[:, c0:c0 + cw])
                nc.vector.tensor_copy(ad_rep[:, c0:c0 + cw], par[:, :cw])

            # ---- AllGather #1: g_nm + as_nm ----
            nc.sync.dma_start(ag1_in[:, 0:640], g_nm_loc[:])
            nc.sync.dma_start(ag1_in[:, 640:640 + NCH], as_nm_loc[:])
            nc.gpsimd.collective_compute(
                "AllGather", mybir.AluOpType.bypass, replica_groups=RG,
                ins=[ag1_in[:]], outs=[ag1_out[:]])
            for r in range(NCORES):
                nc.sync.dma_start(
                    gones[:, r * NCH:(r + 1) * NCH, 0:HID],
                    AP(ag1_out, r * 128 * AG1W,
                       [[AG1W, 128], [HID, NCH], [1, HID]]))
                nc.sync.dma_start(
                    as_nm[:, r * NCH:(r + 1) * NCH],
                    AP(ag1_out, r * 128 * AG1W + 640,
                       [[AG1W, 128], [1, NCH]]))

            # ---- AllReduce for attribute path (overlaps with GAT phase) ----
            nc.gpsimd.collective_compute(
                "AllReduce", mybir.AluOpType.add, replica_groups=RG,
                ins=[ar_in[:]], outs=[ar_out[:]])

            # ---- edge phase ----
            with (
                tc.tile_pool(name="edge", bufs=1) as pe,
                tc.tile_pool(name="rpool", bufs=3) as rp,
                tc.tile_pool(name="psAgg", bufs=1, space="PSUM") as psG,
            ):
                ad_idx_s = pe.tile([128, L // 16], i16)
                sc_idx_s = pe.tile([128, L], i16)
                c1_s = pe.tile([128, L], f32)
                e_mat = pe.tile([128, SB, W], f32)
                p_mat = pe.tile([128, SB, W], bf16)
                nc.sync.dma_start(ad_idx_s[:], adidx_in[:])
                nc.sync.dma_start(sc_idx_s[:], scidx_in[:])
                nc.sync.dma_start(c1_s[:], c1_in[:])

                # gather ad[dst] per edge slot
                e_flat = AP(e_mat[:].tensor, 0, [[L, 128], [1, L], [1, 1]])
                adr3 = AP(ad_rep[:].tensor, ad_rep[:].offset,
                          [[RPC, 128], [1, RPC], [1, 1]])
                nc.gpsimd.ap_gather(e_flat, adr3, ad_idx_s[:],
                                    channels=128, num_elems=RPC, d=1,
                                    num_idxs=L)
                # e += a_s (block-broadcast); lrelu; exp; * multiplicity
                as_bc = AP(as_nm[:].tensor, as_nm[:].offset,
                           [[SB, 128], [1, SB], [0, W]])
                nc.vector.tensor_tensor(e_mat[:], e_mat[:], as_bc, ALU.add)
                nc.vector.scalar_tensor_tensor(
                    e_mat[:], e_mat[:], NEG, e_mat[:], ALU.mult, ALU.max)
                c13 = AP(c1_s[:].tensor, c1_s[:].offset,
                         [[L, 128], [W, SB], [1, W]])
                nc.scalar.activation(e_mat[:], e_mat[:], AF.Exp)
                nc.vector.tensor_tensor(p_mat[:], e_mat[:], c13, ALU.mult)

                # per-src-block: scatter numerators, accumulate [g|1]^T @ R
                pg0 = psG.tile([HID + 1, 512], f32)
                pg1 = psG.tile([HID + 1, 512], f32)
                pg2 = psG.tile([HID + 1, 256], f32)
                pgs = [(pg0, 0, 512), (pg1, 512, 512), (pg2, 1024, 256)]
                for sb in range(SB):
                    r_sb = rp.tile([128, RPC], bf16, tag="r")
                    nc.gpsimd.local_scatter(
                        r_sb[:], p_mat[:, sb, :], sc_idx_s[:, sb * W:(sb + 1) * W],
                        channels=128, num_elems=RPC, num_idxs=W)
                    for pgt, c0, cw in pgs:
                        nc.tensor.matmul(pgt[:, :cw], lhsT=gones[:, sb, :],
                                         rhs=r_sb[:, c0:c0 + cw],
                                         start=(sb == 0), stop=(sb == SB - 1))

                # emb = (weighted sums / denom) + b_gat
                for pgt, c0, cw in pgs:
                    nc.vector.tensor_copy(denom_s[:, c0:c0 + cw],
                                          pgt[HID:HID + 1, :cw])
                nc.vector.reciprocal(denom_r[:], denom_s[:])
                for pgt, c0, cw in pgs:
                    pbr = psA.tile([HID, 512], f32, tag="pa")
                    nc.tensor.matmul(pbr[:, :cw], lhsT=ones_row[0:1, 0:HID],
                                     rhs=denom_r[:, c0:c0 + cw])
                    nc.vector.tensor_copy(recip_bc[:, c0:c0 + cw], pbr[:, :cw])
                    nc.vector.tensor_tensor(emb_T[:, c0:c0 + cw],
                                            pgt[0:HID, :cw],
                                            recip_bc[:, c0:c0 + cw],
                                            ALU.mult)
                nc.vector.tensor_scalar_add(emb_T[:], emb_T[:], bgat[:])
                nc.vector.tensor_copy(emb_bf[:], emb_T[:])

            # ---- AllGather #2: emb ----
            nc.sync.dma_start(ag2_in[:], emb_bf[:])
            nc.gpsimd.collective_compute(
                "AllGather", mybir.AluOpType.bypass, replica_groups=RG,
                ins=[ag2_in[:]], outs=[ag2_out[:]])

            # ---- attribute decoder epilogue (overlaps with s_) ----
            ar_s = pp.tile([EMB, IN], f32)
            a_T = pp.tile([EMB, IN], f32)
            nc.sync.dma_start(ar_s[:], ar_out[:])
            nc.scalar.activation(a_T[:], ar_s[:], AF.Relu, bias=ba1[:])
            pa2 = psA.tile([HID, IN], f32, tag="pa")
            nc.tensor.matmul(pa2[:], lhsT=wa2[:], rhs=a_T[:])
            nc.vector.tensor_scalar_add(a2_T[:], pa2[:], ba2[:])

            with tc.tile_pool(name="phS", bufs=1) as ps_pool:
                emb_full = ps_pool.tile([HID, NPAD], bf16)
                for r in range(NCORES):
                    nc.sync.dma_start(
                        emb_full[:, r * RPC:(r + 1) * RPC],
                        ag2_out[r * HID:(r + 1) * HID, :])

                # x_ = emb @ a2^T  (row-sharded)
                with tc.tile_pool(name="xo", bufs=1) as xop:
                    xo = xop.tile([128, NCH, IN], f32)
                    for ch in range(NCH):
                        px = psA.tile([128, IN], f32, tag="pa")
                        nc.tensor.matmul(px[:],
                                         lhsT=emb_T[:, ch * 128:(ch + 1) * 128],
                                         rhs=a2_T[:])
                        nc.vector.tensor_copy(xo[:, ch, :], px[:])
                    nc.sync.dma_start(
                        AP(out_x, 0, [[IN, 128], [128 * IN, NCH], [1, IN]]),
                        xo[:])

                # s_ = sigmoid(emb_local @ emb_full^T), streamed out
                with (
                    tc.tile_pool(name="sst", bufs=3) as stp,
                    tc.tile_pool(name="psS", bufs=3, space="PSUM") as psS,
                ):
                    for ch in range(NCH):
                        lhs = emb_bf[:, ch * 128:(ch + 1) * 128]
                        for js in range(5):
                            stg = stp.tile([128, 2048], f32, tag="stg")
                            for q in range(4):
                                j0 = js * 2048 + q * 512
                                pss = psS.tile([128, 512], f32, tag="pss")
                                nc.tensor.matmul(
                                    pss[:], lhsT=lhs,
                                    rhs=emb_full[:, j0:j0 + 512])
                                nc.scalar.activation(
                                    stg[:, q * 512:(q + 1) * 512], pss[:],
                                    AF.Sigmoid)
                            ow = min(2048, N - js * 2048)
                            nc.sync.dma_start(
                                AP(out_s, ch * 128 * N + js * 2048,
                                   [[N, 128], [1, ow]]),
                                stg[:, 0:ow])

    nc.compile()
    return nc


def kernel(**inputs):
    import concourse.mybir as mybir  # noqa: F401  (env check)
    from concourse.bass_utils import run_bass_kernel_spmd

    x = np.asarray(inputs["x"], np.float32)
    ei = np.asarray(inputs["edge_index"])
    W_stru = np.asarray(inputs["W_stru"], np.float32)
    b_stru = np.asarray(inputs["b_stru"], np.float32)
    W_gat = np.asarray(inputs["W_gat"], np.float32)
    att_src = np.asarray(inputs["att_src"], np.float32)
    att_dst = np.asarray(inputs["att_dst"], np.float32)
    b_gat = np.asarray(inputs["b_gat"], np.float32)
    W_a1 = np.asarray(inputs["W_a1"], np.float32)
    b_a1 = np.asarray(inputs["b_a1"], np.float32)
    W_a2 = np.asarray(inputs["W_a2"], np.float32)
    b_a2 = np.asarray(inputs["b_a2"], np.float32)

    W, L, ad_idx, sc_idx, c1 = _host_edge_prep(ei)

    key = (W, L)
    if key not in _CACHE:
        _CACHE[key] = _build(W, L)
    nc = _CACHE[key]

    x_pad = np.zeros((NPAD, IN), np.float32)
    x_pad[:N] = x
    wa1T_pad = np.zeros((NPAD, EMB), np.float32)
    wa1T_pad[:N] = W_a1.T
    shared = {
        "W_struT": np.ascontiguousarray(W_stru.T),
        "b_stru": b_stru.reshape(EMB, 1),
        "W_gatT": np.ascontiguousarray(W_gat.T),
        "att2": np.ascontiguousarray(np.stack([att_src, att_dst], 1)),
        "b_gat": b_gat.reshape(HID, 1),
        "b_a1": b_a1.reshape(EMB, 1),
        "W_a2T": np.ascontiguousarray(W_a2.T),
        "b_a2": b_a2.reshape(HID, 1),
        "ident": np.eye(128, dtype=np.float32),
    }
    in_maps = []
    for c in range(NCORES):
        m = dict(shared)
        m["x_c"] = np.ascontiguousarray(x_pad[c * RPC:(c + 1) * RPC])
        m["W_a1T_c"] = np.ascontiguousarray(wa1T_pad[c * RPC:(c + 1) * RPC])
        m["ad_idx"] = np.ascontiguousarray(ad_idx[c])
        m["sc_idx"] = np.ascontiguousarray(sc_idx[c])
        m["c1"] = np.ascontiguousarray(c1[c])
        in_maps.append(m)

    trace = os.environ.get("BASS_KERNEL_TRACE", "0") == "1"
    res = run_bass_kernel_spmd(nc, in_maps, core_ids=list(range(NCORES)),
                               trace=trace)
    if trace and res.exec_time_ns is not None:
        print(f"HW exec time: {res.exec_time_ns} ns")

    s_ = np.concatenate([r["out_s"] for r in res.results], 0)[:N]
    x_ = np.concatenate([r["out_x"] for r in res.results], 0)[:N]
    return (x_, s_)
